# revision 9
# baseline (speedup 1.0000x reference)
"""Trainium2 Bass kernel for nn_ExampleTiedDropout.

out[b, c, h, w] = X[b, c, h, w] * mask[b, c], where mask[b] is a per-example
0/1 channel mask derived deterministically from indices[b]:
  - channels [0, 51) are always kept
  - channels [51, 256) are kept with p=0.1, drawn from
    jax.random.bernoulli(fold_in(key(42), idx))

Strategy: pure data parallel over the batch; each of the 8 cores gets 16
examples. The (tiny) mask generation happens on host exactly as the
reference computes it. Because the mask is 0/1, the output is a sparse
channel COPY: ~28% of channels are verbatim copies of X, the rest are
zero. The device program therefore never reads dropped channels and never
writes them either (output buffers are zero-initialized by the runtime on
both the axon/PJRT path — donated zero buffers — and the native path —
nrt_tensor_write of the zeroed host buffer):
  - the 51 always-kept channels move via direct DRAM->DRAM DMAs on the
    two HWDGE rings,
  - the ~20/example randomly-kept channels move via indirect DMA
    gather->scatter rounds (128 rows of 4KB per round) on the Pool queue,
    with out-of-bounds sentinel padding for unused table slots.
This is ~3.5x less HBM traffic than the dense multiply and measures
~22.5us/core/pass steady-state vs ~98us for the dense version (the
dense program is kept as `_build_program` and used as a fallback).
"""

import numpy as np

B, C, H, W = 128, 256, 32, 32
HW = H * W
P = 128  # SBUF partitions; C == 2 * P
FIXED = 51  # int(0.2 * C): always-kept channels
MEM = C - FIXED
P_MEM = 0.1
N_CORES = 8
BS = B // N_CORES  # examples per core
J = BS * C // P  # [P, HW] tiles per core

_PROG = None


def _compute_masks(indices: np.ndarray) -> np.ndarray:
    """Replicates reference._example_mask exactly: same jax ops, same default
    device, eager (the PRNG impl here is `rbg`, whose draws are
    backend-dependent — so this must run the way the reference runs)."""
    import jax
    import jax.numpy as jnp

    def _example_mask(idx):
        k = jax.random.fold_in(jax.random.key(42), idx)
        mem = jax.random.bernoulli(k, P_MEM, (MEM,)).astype(jnp.float32)
        return jnp.concatenate([jnp.ones((FIXED,), jnp.float32), mem])

    masks = jax.vmap(_example_mask)(jnp.asarray(np.asarray(indices)))
    return np.asarray(masks, dtype=np.float32)


def _build_program(repeat: int = 1, loop_n: int = 0):
    import concourse.bacc as bacc
    import concourse.mybir as mybir
    import concourse.tile as tile

    f32 = mybir.dt.float32
    nc = bacc.Bacc("TRN2", debug=False, num_devices=N_CORES)
    x = nc.dram_tensor("x", [J, P, HW], f32, kind="ExternalInput").ap()
    mt = nc.dram_tensor("mt", [P, J], f32, kind="ExternalInput").ap()
    y = nc.dram_tensor("y", [J, P, HW], f32, kind="ExternalOutput").ap()

    with tile.TileContext(nc) as tc:
        with (
            tc.tile_pool(name="msk", bufs=1) as mpool,
            tc.tile_pool(name="xin", bufs=6) as xpool,
            tc.tile_pool(name="yout", bufs=6) as ypool,
        ):
            m = mpool.tile([P, J], f32)
            nc.sync.dma_start(m[:], mt[:])

            def one_pass():
                for j in range(J):
                    t = xpool.tile([P, HW], f32)
                    nc.sync.dma_start(t[:], x[j])
                    o = ypool.tile([P, HW], f32)
                    nc.vector.tensor_scalar_mul(o[:], t[:], m[:, j : j + 1])
                    nc.scalar.dma_start(y[j], o[:])

            if loop_n:
                with tc.For_i(0, loop_n, 1):
                    for _ in range(repeat):
                        one_pass()
            else:
                for _ in range(repeat):
                    one_pass()
    nc.compile()
    return nc


MAX_ROUNDS = 20  # beyond this the dense program is competitive
SENTINEL = 1 << 20  # > bounds_check => row skipped by the DMA engine


def _build_sparse_program(rounds: int = 3, loop_n: int = 0):
    """Copy-only kernel: always-kept channel block via one direct DRAM->DRAM
    DMA; randomly-kept channels via indirect gather->scatter rounds (one row
    of 4KB per partition). Dropped channels are never written: the output
    buffer is zero-initialized by the runtime (donated zero buffers)."""
    import concourse.bacc as bacc
    import concourse.mybir as mybir
    import concourse.tile as tile

    f32 = mybir.dt.float32
    i32 = mybir.dt.int32
    NR = BS * C  # 4096 rows per core
    nc = bacc.Bacc("TRN2", debug=False, num_devices=N_CORES)
    x = nc.dram_tensor("x", [NR, HW], f32, kind="ExternalInput").ap()
    tbl = nc.dram_tensor("tbl", [P, rounds], i32, kind="ExternalInput").ap()
    y = nc.dram_tensor("y", [NR, HW], f32, kind="ExternalOutput").ap()

    x4 = x.rearrange("(b c) m -> b c m", c=C)
    y4 = y.rearrange("(b c) m -> b c m", c=C)

    with tile.TileContext(nc) as tc:
        with (
            tc.tile_pool(name="tblp", bufs=1) as tpool,
            tc.tile_pool(name="gat", bufs=3) as gpool,
        ):
            tb = tpool.tile([P, rounds], i32)
            nc.sync.dma_start(tb[:], tbl[:])

            def one_pass():
                # fixed block: channels [0, FIXED) of every example, split
                # over both HWDGE rings
                h = BS // 2
                nc.sync.dma_start(y4[:h, 0:FIXED, :], x4[:h, 0:FIXED, :])
                nc.scalar.dma_start(y4[h:, 0:FIXED, :], x4[h:, 0:FIXED, :])
                # random kept channels: gather rounds of 128 rows, then
                # scatter them back. All gathers are emitted before any
                # scatter: interleaving stalls the Pool engine on each
                # gather's DMA-completion semaphore before it can generate
                # the next round's descriptors (~3.5us/instruction vs
                # ~0.85us in this order).
                import concourse.bass as bass

                gs = []
                for r in range(rounds):
                    g = gpool.tile([P, HW], f32, tag=f"g{r}")
                    gs.append(g)
                    nc.gpsimd.indirect_dma_start(
                        out=g[:],
                        out_offset=None,
                        in_=x[:, :],
                        in_offset=bass.IndirectOffsetOnAxis(
                            ap=tb[:, r : r + 1], axis=0
                        ),
                        bounds_check=NR - 1,
                        oob_is_err=False,
                    )
                for r in range(rounds):
                    nc.gpsimd.indirect_dma_start(
                        out=y[:, :],
                        out_offset=bass.IndirectOffsetOnAxis(
                            ap=tb[:, r : r + 1], axis=0
                        ),
                        in_=gs[r][:],
                        in_offset=None,
                        bounds_check=NR - 1,
                        oob_is_err=False,
                    )

            if loop_n:
                with tc.For_i(0, loop_n, 1):
                    one_pass()
            else:
                one_pass()
    nc.compile()
    return nc


def _make_tables(masks: np.ndarray):
    """Per-core [P, rounds] int32 row-index tables for the randomly-kept
    channels (global row = b_local*C + c), SENTINEL-padded. Returns
    (tables, rounds), or None if the dense fallback should be used."""
    per_core_rows = []
    for i in range(N_CORES):
        rows = []
        mk = masks[i * BS : (i + 1) * BS]  # [BS, C]
        for b in range(BS):
            (kept,) = np.nonzero(mk[b, FIXED:] != 0.0)
            rows.extend(b * C + FIXED + kept)
        per_core_rows.append(rows)
    rounds = max(1, -(-max(len(r) for r in per_core_rows) // P))
    if rounds > MAX_ROUNDS:
        return None
    tables = []
    for rows in per_core_rows:
        flat = np.full(rounds * P, SENTINEL, dtype=np.int32)
        flat[: len(rows)] = np.asarray(rows, dtype=np.int32)
        tables.append(np.ascontiguousarray(flat.reshape(rounds, P).T))
    return tables, rounds


def kernel(X: np.ndarray, indices: np.ndarray) -> np.ndarray:
    from concourse.bass_utils import run_bass_kernel_spmd

    global _PROG
    X = np.ascontiguousarray(np.asarray(X, dtype=np.float32))
    masks = _compute_masks(indices)  # [B, C] float32
    made = _make_tables(masks)

    if made is not None:
        tables, rounds = made
        if _PROG is None or _PROG[0] != ("sparse", rounds):
            _PROG = (("sparse", rounds), _build_sparse_program(rounds))
        nc = _PROG[1]
        in_maps = [
            {"x": X[i * BS : (i + 1) * BS].reshape(BS * C, HW), "tbl": tables[i]}
            for i in range(N_CORES)
        ]
    else:  # huge kept fraction (practically impossible): dense fallback
        if _PROG is None or _PROG[0] != "dense":
            _PROG = ("dense", _build_program())
        nc = _PROG[1]
        in_maps = []
        for i in range(N_CORES):
            xs = X[i * BS : (i + 1) * BS].reshape(J, P, HW)
            ms = np.ascontiguousarray(masks[i * BS : (i + 1) * BS].reshape(J, P).T)
            in_maps.append({"x": xs, "mt": ms})

    res = run_bass_kernel_spmd(nc, in_maps, core_ids=list(range(N_CORES)))
    out = np.concatenate(
        [res.results[i]["y"].reshape(BS, C, H, W) for i in range(N_CORES)], axis=0
    )
    return out
